# revision 21
# baseline (speedup 1.0000x reference)
"""Trainium2 Bass kernel for a vanilla tanh RNN:
    xp = x @ Wxh + b                      # [B, T, H] input projection
    h_t = tanh(xp_t + h_{t-1} @ Whh)      # returns h_{T-1}  [B, H]

Shapes: B=256, T=256, D=1024, H=1024 fp32.
Sharding: data-parallel over 8 cores, batch split 32/core, weights replicated.

Per-core design (single fused instruction stream):
  * h is kept TRANSPOSED: hT = 8 chunks of [128(h), 32(b)] fp16, ping-pong.
    Recurrence matmuls use Whh fp16 tiles as the stationary operand:
      psum[mc] += Whh16[kc, mc].T @ hT[kc]   (out lands in hT layout directly,
    so there is NO per-step transpose).
  * Per step, output-chunk group mc issues its 8 contraction matmuls in
    ROTATED order kc = (mc+1+j) % 8.  This bounds (last-accumulation
    position of chunk mc in step t) - (first-use position in step t+1) by
    ~15 matmul slots, which hides the DVE-add + ACT-tanh epilogue latency
    completely: the PE never stalls between steps.
  * The xp GEMM is interleaved INTO the recurrence stream 2 groups (32
    timesteps) ahead, 8 matmuls every other step; its input pipeline
    (fp32->fp16 cast, DMA-transpose to put D on partitions) runs 2-3 groups
    ahead on DMA/DVE.  xp stays in SBUF (no DRAM roundtrip).
  * Epilogue per step/chunk: DVE add (psum + xp), ACT tanh (+bias, fp16 out).
  * Tail: PE-transpose of the final fp32 h into batch-major and one
    contiguous output DMA (avoids 4-byte scatter DMA).
"""

import os

import numpy as np

import concourse.bass as bass
import concourse.mybir as mybir
import concourse.tile as tile
from concourse import bacc
from concourse._compat import axon_active
from concourse.bass_utils import run_bass_kernel_spmd
from concourse.masks import make_identity

F32 = mybir.dt.float32
F16 = mybir.dt.float16

B, T, D, H = 256, 256, 1024, 1024
NCORES = 8
BL = B // NCORES  # 32 batch per core
P = 128
KC = H // P  # 8 contraction chunks for Whh
KD = D // P  # 8 contraction chunks for Wxh
MC = H // P  # 8 output chunks
TanhF = mybir.ActivationFunctionType.Tanh
CopyF = mybir.ActivationFunctionType.Copy

NBT = 512            # bt elements per GEMM group (16 t x 32 b, t-major)
TG = NBT // BL       # 16 timesteps per group
NG = T // TG         # 16 groups


def _build():
    nc = bacc.Bacc(
        os.environ.get("TRN_TYPE") or "TRN2",
        target_bir_lowering=False,
        debug=not axon_active(),
    )

    x_t = nc.dram_tensor("xT", [D, T, BL], F16, kind="ExternalInput")
    wxh_t = nc.dram_tensor("Wxh", [D, H], F16, kind="ExternalInput")
    whh_t = nc.dram_tensor("Whh", [H, H], F16, kind="ExternalInput")
    b_t = nc.dram_tensor("b", [H], F32, kind="ExternalInput")
    out_t = nc.dram_tensor("h_out", [BL, H], F32, kind="ExternalOutput")

    with tile.TileContext(nc) as tc:
        with (
            tc.tile_pool(name="const", bufs=1) as const,
            tc.tile_pool(name="xin", bufs=4) as xinp,
            tc.tile_pool(name="xpp", bufs=3) as xpp,
            tc.tile_pool(name="psum_r", bufs=3, space="PSUM") as psum_r,
            tc.tile_pool(name="psum_g", bufs=4, space="PSUM") as psum_g,
        ):
            # ---- constants: identity, x group-0 loads first (DMA priority) ----
            ident = const.tile([P, P], F32, tag="ident")
            make_identity(nc, ident[:])
            ident16 = const.tile([P, P], F16, tag="ident16")
            make_identity(nc, ident16[:])

            def emit_loads(g):
                """Load host-transposed fp16 x slices: 8 [128(d), TG, BL] tiles."""
                tiles = []
                for k in range(KD):
                    xin = xinp.tile([P, TG, BL], F16, tag=f"xin{k}", name=f"xin{k}")
                    nc.sync.dma_start(
                        xin[:], x_t.ap()[k * P : (k + 1) * P, g * TG : (g + 1) * TG, :]
                    )
                    tiles.append(xin)
                return tiles

            # weights are fp16 on host already: straight DMA, per k-chunk;
            # first GEMM matmul needs LDW(wxh k=0) so that DMA goes first
            wxh16 = const.tile([P, KD, H], F16, tag="wxh16")
            whh16 = const.tile([P, KC, H], F16, tag="whh16")
            nc.sync.dma_start(wxh16[:, 0, :], wxh_t.ap()[0:P, :])
            xin_t, xp_t = {}, {}
            xin_t[0] = emit_loads(0)
            for k in range(1, KD):
                nc.sync.dma_start(
                    wxh16[:, k, :], wxh_t.ap()[k * P : (k + 1) * P, :]
                )
            # bias as per-partition column vector [P, MC]; folded into the
            # xp psum evacuation on DVE
            bcol = const.tile([P, MC], F32, tag="bcol")
            nc.sync.dma_start(bcol[:], b_t.ap().rearrange("(mc p) -> p mc", p=P))

            def alloc_xp():
                return xpp.tile([P, TG, MC, BL], F16, tag="xpq", name="xpq")

            def gemm_ops(xT, xp_tiles):
                """Flat op list for one group's xp GEMM: per output chunk mc,
                8 contraction MMs + DVE evac (adds bias) into the quad xp tile."""
                ops = []
                state = {}

                def mk_mm(mc, k):
                    def run():
                        if k == 0:
                            state[mc] = psum_g.tile(
                                [P, NBT], F32, tag="psg", name="psg"
                            )
                        nc.tensor.matmul(
                            state[mc][:],
                            wxh16[:, k, mc * P : (mc + 1) * P],
                            xT[k][:].rearrange("p t b -> p (t b)"),
                            start=(k == 0),
                            stop=(k == KD - 1),
                        )
                    return run

                def mk_evac(mc, half):
                    def run():
                        hh = slice(half * (TG // 2), (half + 1) * (TG // 2))
                        nc.vector.tensor_scalar_add(
                            xp_tiles[:, hh, mc, :],
                            state[mc][:].rearrange("p (t b) -> p t b", b=BL)[
                                :, hh, :
                            ],
                            bcol[:, mc : mc + 1],
                        )
                    return run

                for mc in range(MC):
                    for k in range(KD):
                        ops.append(mk_mm(mc, k))
                    ops.append(mk_evac(mc, 0))
                    ops.append(mk_evac(mc, 1))
                return ops

            # ---- recurrence state ----
            hbuf = [
                [
                    const.tile([P, 4, BL], F16, tag=f"h{i}_{q}", name=f"h{i}_{q}")
                    for q in range(2)
                ]
                for i in range(2)
            ]
            for q in range(2):
                nc.vector.memset(hbuf[0][q][:], 0.0)
            h32 = const.tile([P, MC, BL], F32, tag="h32")

            # ---- prologue: group-0 GEMM first, whh loads behind it ----
            xp_t[0] = alloc_xp()
            for op in gemm_ops(xin_t[0], xp_t[0]):
                op()
            for k in range(KC):
                nc.sync.dma_start(
                    whh16[:, k, :], whh_t.ap()[k * P : (k + 1) * P, :]
                )
            for g in range(1, min(4, NG)):
                xin_t[g] = emit_loads(g)

            # ---- main loop: recurrence with interleaved xp production ----
            for g in range(NG):
                if g + 4 < NG:
                    xin_t[g + 4] = emit_loads(g + 4)
                pending = []
                if g == 0:
                    xp_t[1] = alloc_xp()
                    pending += gemm_ops(xin_t[1], xp_t[1])
                if g + 2 < NG - 1:
                    xp_t[g + 2] = alloc_xp()
                    pending += gemm_ops(xin_t[g + 2], xp_t[g + 2])
                elif g == NG - 3:
                    # split the last group's GEMM over g13/g14 so the tail
                    # isn't filler-dry: mc 0..3 here, mc 4..7 next group
                    xp_t[NG - 1] = alloc_xp()
                    gemm15 = gemm_ops(xin_t[NG - 1], xp_t[NG - 1])
                    pending += gemm15[: 4 * (KD + 2)]
                elif g == NG - 2:
                    pending += gemm15[4 * (KD + 2) :]
                cursor = 0
                for lt in range(TG):
                    t = g * TG + lt
                    src = hbuf[t % 2]
                    dst = hbuf[1 - t % 2]
                    psq = psum_r.tile([P, MC, BL], F32, tag="psr", name="psr")
                    # inject xp_t (+b, already folded) into PSUM with a single
                    # N=256 ident matmul; no h dependency
                    nc.tensor.matmul(
                        psq[:],
                        ident16[:],
                        xp_t[g][:, lt, :, :],
                        start=True,
                        stop=False,
                        skip_group_check=True,
                    )
                    # Half-block schedule. OA/OB = output quads (psq[0]/[1]),
                    # IA/IB = input h quads (src[0]/[1]).  Order:
                    #   [OA*IA 16][OA*IB 8][OB*IA 8][OA*IB 8] ACT(A)@42
                    #   [OB*IA 8][OB*IB 16] ACT(B)@66
                    # -> next step consumes IA at slot 3 (slack 28) and IB at
                    # slot 27 (slack 28): ~700ns for each tanh to land.
                    def rec_mm(mc, kc, iq, stop):
                        nc.tensor.matmul(
                            psq[:, mc, :],
                            whh16[:, kc, mc * P : (mc + 1) * P],
                            src[iq][:, kc % 4, :],
                            start=False,
                            stop=stop,
                            skip_group_check=True,
                        )

                    def act(q):
                        qq = slice(q * 4, (q + 1) * 4)
                        if t < T - 1:
                            nc.scalar.activation(dst[q][:], psq[:, qq, :], TanhF)
                        else:
                            nc.scalar.activation(
                                h32[:, qq, :], psq[:, qq, :], TanhF
                            )

                    for mc in (0, 1, 2, 3):
                        for j in range(4):
                            rec_mm(mc, (mc + j) % 4, 0, False)
                    for mc in (0, 1):
                        for j in range(4):
                            rec_mm(mc, 4 + (mc + j) % 4, 1, j == 3)
                    for mc in (4, 5):
                        for j in range(4):
                            rec_mm(mc, (mc + j) % 4, 0, False)
                    for mc in (2, 3):
                        for j in range(4):
                            rec_mm(mc, 4 + (mc + j) % 4, 1, j == 3)
                    act(0)
                    for mc in (6, 7):
                        for j in range(4):
                            rec_mm(mc, (mc + j) % 4, 0, False)
                    for mc in (4, 5, 6, 7):
                        for j in range(4):
                            rec_mm(mc, 4 + (mc + j) % 4, 1, j == 3)
                    act(1)
                    # spread this group's (g+2) xp GEMM uniformly over the steps
                    target = (lt + 1) * len(pending) // TG
                    while cursor < target:
                        pending[cursor]()
                        cursor += 1


            # ---- tail: transpose h32 to batch-major, contiguous output DMA ----
            h_final = const.tile([BL, H], F32, tag="h_final")
            for mc in range(MC):
                pst = psum_g.tile([BL, P], F32, tag="psg", name="pst")
                nc.tensor.transpose(pst[:], h32[:, mc, :], ident[:])
                nc.vector.tensor_copy(
                    h_final[:, mc * P : (mc + 1) * P], pst[:]
                )
            nc.sync.dma_start(out_t.ap(), h_final[:])

    nc.compile()
    return nc


_nc = None
last_results = None


def kernel(x, Wxh, Whh, b):
    global _nc, last_results
    if _nc is None:
        _nc = _build()
    # host-side fp16 cast + transpose: xT[d, t, b] = x[b, t, d]; sharded on b
    xT = np.ascontiguousarray(
        np.asarray(x, dtype=np.float32).astype(np.float16).transpose(2, 1, 0)
    )
    Wxh = np.asarray(Wxh, dtype=np.float32).astype(np.float16)
    Whh = np.asarray(Whh, dtype=np.float32).astype(np.float16)
    b = np.asarray(b, dtype=np.float32)
    in_maps = [
        {
            "xT": np.ascontiguousarray(xT[:, :, c * BL : (c + 1) * BL]),
            "Wxh": Wxh,
            "Whh": Whh,
            "b": b,
        }
        for c in range(NCORES)
    ]
    last_results = run_bass_kernel_spmd(_nc, in_maps, list(range(NCORES)))
    out = np.concatenate(
        [last_results.results[c]["h_out"] for c in range(NCORES)], axis=0
    )
    return out



# revision 23
# speedup vs baseline: 1.1225x; 1.1225x over previous
"""Trainium2 Bass kernel for a vanilla tanh RNN:
    xp = x @ Wxh + b                      # [B, T, H] input projection
    h_t = tanh(xp_t + h_{t-1} @ Whh)      # returns h_{T-1}  [B, H]

Shapes: B=256, T=256, D=1024, H=1024 fp32.
Sharding: data-parallel over 8 cores, batch split 32/core, weights replicated.

Per-core design (single fused instruction stream):
  * h is kept TRANSPOSED: hT = 8 chunks of [128(h), 32(b)] fp16, ping-pong.
    Recurrence matmuls use Whh fp16 tiles as the stationary operand:
      psum[mc] += Whh16[kc, mc].T @ hT[kc]   (out lands in hT layout directly,
    so there is NO per-step transpose).
  * Per step, output-chunk group mc issues its 8 contraction matmuls in
    ROTATED order kc = (mc+1+j) % 8.  This bounds (last-accumulation
    position of chunk mc in step t) - (first-use position in step t+1) by
    ~15 matmul slots, which hides the DVE-add + ACT-tanh epilogue latency
    completely: the PE never stalls between steps.
  * The xp GEMM is interleaved INTO the recurrence stream 2 groups (32
    timesteps) ahead, 8 matmuls every other step; its input pipeline
    (fp32->fp16 cast, DMA-transpose to put D on partitions) runs 2-3 groups
    ahead on DMA/DVE.  xp stays in SBUF (no DRAM roundtrip).
  * Epilogue per step/chunk: DVE add (psum + xp), ACT tanh (+bias, fp16 out).
  * Tail: PE-transpose of the final fp32 h into batch-major and one
    contiguous output DMA (avoids 4-byte scatter DMA).
"""

import os

import numpy as np

import concourse.bass as bass
import concourse.mybir as mybir
import concourse.tile as tile
from concourse import bacc
from concourse._compat import axon_active
from concourse.bass_utils import run_bass_kernel_spmd
from concourse.masks import make_identity

F32 = mybir.dt.float32
F16 = mybir.dt.float16

B, T, D, H = 256, 256, 1024, 1024
NCORES = 8
BL = B // NCORES  # 32 batch per core
P = 128
KC = H // P  # 8 contraction chunks for Whh
KD = D // P  # 8 contraction chunks for Wxh
MC = H // P  # 8 output chunks
TanhF = mybir.ActivationFunctionType.Tanh
CopyF = mybir.ActivationFunctionType.Copy

NBT = 512            # bt elements per GEMM group (16 t x 32 b, t-major)
TG = NBT // BL       # 16 timesteps per group
NG = T // TG         # 16 groups


def _build():
    nc = bacc.Bacc(
        os.environ.get("TRN_TYPE") or "TRN2",
        target_bir_lowering=False,
        debug=not axon_active(),
    )

    x_t = nc.dram_tensor("xT", [D, T, BL], F16, kind="ExternalInput")
    wxh_t = nc.dram_tensor("Wxh", [D, H], F16, kind="ExternalInput")
    whh_t = nc.dram_tensor("Whh", [H, H], F16, kind="ExternalInput")
    b_t = nc.dram_tensor("b", [H], F32, kind="ExternalInput")
    out_t = nc.dram_tensor("h_out", [BL, H], F32, kind="ExternalOutput")

    with tile.TileContext(nc) as tc:
        with (
            tc.tile_pool(name="const", bufs=1) as const,
            tc.tile_pool(name="xin", bufs=4) as xinp,
            tc.tile_pool(name="xpp", bufs=3) as xpp,
            tc.tile_pool(name="psum_r", bufs=3, space="PSUM") as psum_r,
            tc.tile_pool(name="psum_g", bufs=4, space="PSUM") as psum_g,
        ):
            # ---- constants: identity, x group-0 loads first (DMA priority) ----
            ident = const.tile([P, P], F32, tag="ident")
            make_identity(nc, ident[:])
            ident16 = const.tile([P, P], F16, tag="ident16")
            make_identity(nc, ident16[:])

            def emit_loads(g):
                """Load host-transposed fp16 x slices: 8 [128(d), TG, BL] tiles."""
                tiles = []
                for k in range(KD):
                    xin = xinp.tile([P, TG, BL], F16, tag=f"xin{k}", name=f"xin{k}")
                    nc.sync.dma_start(
                        xin[:], x_t.ap()[k * P : (k + 1) * P, g * TG : (g + 1) * TG, :]
                    )
                    tiles.append(xin)
                return tiles

            # weights are fp16 on host already: straight DMA, per k-chunk;
            # first GEMM matmul needs LDW(wxh k=0) so that DMA goes first
            wxh16 = const.tile([P, KD, H], F16, tag="wxh16")
            whh16 = const.tile([P, KC, H], F16, tag="whh16")
            nc.sync.dma_start(wxh16[:, 0, :], wxh_t.ap()[0:P, :])
            xin_t, xp_t = {}, {}
            xin_t[0] = emit_loads(0)
            for k in range(1, KD):
                nc.sync.dma_start(
                    wxh16[:, k, :], wxh_t.ap()[k * P : (k + 1) * P, :]
                )
            # bias as per-partition column vector [P, MC]; folded into the
            # xp psum evacuation on DVE
            bcol = const.tile([P, MC], F32, tag="bcol")
            nc.sync.dma_start(bcol[:], b_t.ap().rearrange("(mc p) -> p mc", p=P))

            def alloc_xp():
                return xpp.tile([P, TG, MC, BL], F16, tag="xpq", name="xpq")

            def gemm_ops(xT, xp_tiles):
                """Flat op list for one group's xp GEMM: per output chunk mc,
                8 contraction MMs + DVE evac (adds bias) into the quad xp tile."""
                ops = []
                state = {}

                def mk_mm(mc, k):
                    def run():
                        if k == 0:
                            state[mc] = psum_g.tile(
                                [P, NBT], F32, tag="psg", name="psg"
                            )
                        nc.tensor.matmul(
                            state[mc][:],
                            wxh16[:, k, mc * P : (mc + 1) * P],
                            xT[k][:].rearrange("p t b -> p (t b)"),
                            start=(k == 0),
                            stop=(k == KD - 1),
                        )
                    return run

                def mk_evac(mc, half):
                    def run():
                        hh = slice(half * (TG // 2), (half + 1) * (TG // 2))
                        nc.vector.tensor_scalar_add(
                            xp_tiles[:, hh, mc, :],
                            state[mc][:].rearrange("p (t b) -> p t b", b=BL)[
                                :, hh, :
                            ],
                            bcol[:, mc : mc + 1],
                        )
                    return run

                for mc in range(MC):
                    for k in range(KD):
                        ops.append(mk_mm(mc, k))
                    ops.append(mk_evac(mc, 0))
                    ops.append(mk_evac(mc, 1))
                return ops

            # ---- recurrence state ----
            hbuf = [
                [
                    const.tile([P, 4, BL], F16, tag=f"h{i}_{q}", name=f"h{i}_{q}")
                    for q in range(2)
                ]
                for i in range(2)
            ]
            for q in range(2):
                nc.vector.memset(hbuf[0][q][:], 0.0)
            h32 = const.tile([P, MC, BL], F32, tag="h32")

            # ---- prologue: group-0 GEMM first, whh loads behind it ----
            xp_t[0] = alloc_xp()
            for op in gemm_ops(xin_t[0], xp_t[0]):
                op()
            for k in range(KC):
                nc.sync.dma_start(
                    whh16[:, k, :], whh_t.ap()[k * P : (k + 1) * P, :]
                )
            for g in range(1, min(4, NG)):
                xin_t[g] = emit_loads(g)

            # ---- main loop: recurrence with interleaved xp production ----
            for g in range(NG):
                if g + 4 < NG:
                    xin_t[g + 4] = emit_loads(g + 4)
                pending = []
                if g == 0:
                    xp_t[1] = alloc_xp()
                    pending += gemm_ops(xin_t[1], xp_t[1])
                if g + 2 < NG - 1:
                    xp_t[g + 2] = alloc_xp()
                    pending += gemm_ops(xin_t[g + 2], xp_t[g + 2])
                elif g == NG - 3:
                    # split the last group's GEMM over g13/g14 so the tail
                    # isn't filler-dry: mc 0..3 here, mc 4..7 next group
                    xp_t[NG - 1] = alloc_xp()
                    gemm15 = gemm_ops(xin_t[NG - 1], xp_t[NG - 1])
                    pending += gemm15[: 4 * (KD + 2)]
                elif g == NG - 2:
                    pending += gemm15[4 * (KD + 2) :]
                cursor = 0
                for lt in range(TG):
                    t = g * TG + lt
                    src = hbuf[t % 2]
                    dst = hbuf[1 - t % 2]
                    psq = [
                        psum_r.tile(
                            [P, 4, BL], F32, tag=f"psr{q}", name="psr", bufs=2
                        )
                        for q in range(2)
                    ]
                    # inject xp_t (+b, already folded) into PSUM; no h
                    # dependency, and one ident LDW serves both quads
                    for q in range(2):
                        nc.tensor.matmul(
                            psq[q][:],
                            ident16[:],
                            xp_t[g][:, lt, q * 4 : (q + 1) * 4, :],
                            start=True,
                            stop=False,
                            skip_group_check=True,
                        )
                    # Half-block schedule. OA/OB = output quads (psq[0]/[1]),
                    # IA/IB = input h quads (src[0]/[1]).  Order:
                    #   [OA*IA 16][OA*IB 8][OB*IA 8][OA*IB 8] ACT(A)@42
                    #   [OB*IA 8][OB*IB 16] ACT(B)@66
                    # -> next step consumes IA at slot 3 (slack 28) and IB at
                    # slot 27 (slack 28): ~700ns for each tanh to land.
                    def rec_mm(mc, kc, iq, stop):
                        nc.tensor.matmul(
                            psq[mc // 4][:, mc % 4, :],
                            whh16[:, kc, mc * P : (mc + 1) * P],
                            src[iq][:, kc % 4, :],
                            start=False,
                            stop=stop,
                            skip_group_check=True,
                        )

                    def act(q):
                        if t < T - 1:
                            nc.scalar.activation(dst[q][:], psq[q][:], TanhF)
                        else:
                            nc.scalar.activation(
                                h32[:, q * 4 : (q + 1) * 4, :], psq[q][:], TanhF
                            )

                    for mc in (0, 1, 2, 3):
                        for j in range(4):
                            rec_mm(mc, (mc + j) % 4, 0, False)
                    for mc in (0, 1):
                        for j in range(4):
                            rec_mm(mc, 4 + (mc + j) % 4, 1, j == 3)
                    for mc in (4, 5):
                        for j in range(4):
                            rec_mm(mc, (mc + j) % 4, 0, False)
                    for mc in (2, 3):
                        for j in range(4):
                            rec_mm(mc, 4 + (mc + j) % 4, 1, j == 3)
                    act(0)
                    for mc in (6, 7):
                        for j in range(4):
                            rec_mm(mc, (mc + j) % 4, 0, False)
                    for mc in (4, 5, 6, 7):
                        for j in range(4):
                            rec_mm(mc, 4 + (mc + j) % 4, 1, j == 3)
                    act(1)
                    # spread this group's (g+2) xp GEMM uniformly over the steps
                    target = (lt + 1) * len(pending) // TG
                    while cursor < target:
                        pending[cursor]()
                        cursor += 1


            # ---- tail: transpose h32 to batch-major, contiguous output DMA ----
            h_final = const.tile([BL, H], F32, tag="h_final")
            for mc in range(MC):
                pst = psum_g.tile([BL, P], F32, tag="psg", name="pst")
                nc.tensor.transpose(pst[:], h32[:, mc, :], ident[:])
                nc.vector.tensor_copy(
                    h_final[:, mc * P : (mc + 1) * P], pst[:]
                )
            nc.sync.dma_start(out_t.ap(), h_final[:])

    nc.compile()
    return nc


_nc = None
last_results = None


def kernel(x, Wxh, Whh, b):
    global _nc, last_results
    if _nc is None:
        _nc = _build()
    # host-side fp16 cast + transpose: xT[d, t, b] = x[b, t, d]; sharded on b
    xT = np.ascontiguousarray(
        np.asarray(x, dtype=np.float32).astype(np.float16).transpose(2, 1, 0)
    )
    Wxh = np.asarray(Wxh, dtype=np.float32).astype(np.float16)
    Whh = np.asarray(Whh, dtype=np.float32).astype(np.float16)
    b = np.asarray(b, dtype=np.float32)
    in_maps = [
        {
            "xT": np.ascontiguousarray(xT[:, :, c * BL : (c + 1) * BL]),
            "Wxh": Wxh,
            "Whh": Whh,
            "b": b,
        }
        for c in range(NCORES)
    ]
    last_results = run_bass_kernel_spmd(_nc, in_maps, list(range(NCORES)))
    out = np.concatenate(
        [last_results.results[c]["h_out"] for c in range(NCORES)], axis=0
    )
    return out

